# revision 3
# baseline (speedup 1.0000x reference)
"""Trainium2 Bass kernel for nn_AttentionHead (B=4, T=2048, D=1024, H=16).

Math shortcut (exact, validated vs reference):
  pooled[b] = (concat_h[ (w*r_h)^T E_h V_h ] + bv) @ Wo + bo
where E_h = exp(Q_h K_h^T / 8) (no max-subtraction needed: |scores| < ~3),
r = 1/rowsum(E), w[t] = (1/(H*T)) sum_{h,q} E_h[q,t] r_q  (head-avg column
sums of softmax), so the full attn@V [B,H,T,T]x[T,HD] and the [B*T,D]@Wo
matmuls are never materialized.

Sharding: 8 cores = (batch b = core//2) x (head-group g = core%2, 8 heads
each). w mixes all 16 heads of a batch -> one tiny [2048] f32 AllReduce
between core pairs mid-kernel. Host sums the two per-batch partial outputs
and adds the exact bias correction bv@Wo + bo.
"""

import os
import sys

for _p in ("/opt/trn_rl_repo",):
    if _p not in sys.path and os.path.isdir(_p):
        sys.path.insert(0, _p)

from contextlib import ExitStack

import numpy as np

import concourse.bass as bass
import concourse.mybir as mybir
import concourse.tile as tile
from concourse import bacc
from concourse.bass_utils import run_bass_kernel_spmd
from concourse.masks import make_identity

FP32 = mybir.dt.float32
BF16 = mybir.dt.bfloat16
AF = mybir.ActivationFunctionType

P = 128
B, T, D, H = 4, 2048, 1024, 16
HD = D // H          # 64
NH = 8               # heads per core
NHD = NH * HD        # 512 cols per core
TQ = T // P          # 16 q-chunks
MC = D // P          # 8 contraction chunks for projections
E_BYTES = 2


def _body(tc, x_d, wq_d, wk_d, wv_d, wo_d, bqs_d, bkc_d, out_d):
    nc = tc.nc
    with ExitStack() as ctx:
        pers = ctx.enter_context(tc.tile_pool(name="pers", bufs=1))

        def ptile(shape, dtype, name):
            return pers.tile(shape, dtype, name=name, tag=name)

        QT = [ptile([P, T], BF16, f"QT{i}") for i in range(4)]
        KT = [ptile([P, T], BF16, f"KT{i}") for i in range(4)]
        Vt = [ptile([P, NHD], BF16, f"V{i}") for i in range(TQ)]
        wo_bf = [ptile([P, D], BF16, f"wo{i}") for i in range(4)]
        Zh = [ptile([P, 2 * TQ], FP32, f"Z{h}") for h in range(NH)]
        rV = [ptile([P, TQ], FP32, f"rV{h}") for h in range(NH)]
        rB = [ptile([P, TQ], BF16, f"rB{h}") for h in range(NH)]
        gB = [ptile([P, TQ], BF16, f"gB{h}") for h in range(NH)]
        w_col = ptile([P, TQ], FP32, "w_col")
        biasq = ptile([P, 4], FP32, "biasq")
        biask = ptile([P, 4], FP32, "biask")
        ident = ptile([P, P], FP32, "ident")
        c_sb = ptile([1, T], FP32, "c_sb")

        make_identity(nc, ident)
        nc.sync.dma_start(biasq, bqs_d.rearrange("(c p) -> p c", p=P))
        nc.sync.dma_start(biask, bkc_d.rearrange("(c p) -> p c", p=P))

        dram = ctx.enter_context(tc.tile_pool(name="dram", bufs=1, space="DRAM"))
        E_spill = dram.tile([NH * T, T], BF16, name="E_spill", tag="E_spill")
        c_bounce = dram.tile([1, T], FP32, name="c_bounce", tag="c_bounce")
        w_bounce = dram.tile([1, T], FP32, name="w_bounce", tag="w_bounce")
        u_dram = dram.tile([NH, T], BF16, name="u_dram", tag="u_dram")
        pooled_dram = dram.tile([1, NHD], BF16, name="pooled_dram", tag="pooled_dram")

        # ---------------- phase 1-3: xT, weights, Q/K/V projections ------
        with ExitStack() as ph:
            stage = ph.enter_context(tc.tile_pool(name="stage", bufs=4))
            xT = [ph.enter_context(tc.tile_pool(name=f"xTp{m}", bufs=1)).tile(
                [P, T], BF16, name=f"xT{m}", tag=f"xT{m}") for m in range(MC)]
            wq_bf = [ph.enter_context(tc.tile_pool(name=f"wqp{m}", bufs=1)).tile(
                [P, NHD], BF16, name=f"wq{m}", tag=f"wq{m}") for m in range(MC)]
            wk_bf = [ph.enter_context(tc.tile_pool(name=f"wkp{m}", bufs=1)).tile(
                [P, NHD], BF16, name=f"wk{m}", tag=f"wk{m}") for m in range(MC)]
            wv_bf = [ph.enter_context(tc.tile_pool(name=f"wvp{m}", bufs=1)).tile(
                [P, NHD], BF16, name=f"wv{m}", tag=f"wv{m}") for m in range(MC)]
            psA = ph.enter_context(tc.tile_pool(name="psA", bufs=2, space="PSUM"))

            # x -> xT (PE transpose, f32 -> bf16 on eviction)
            for grp in range(4):
                xs = []
                for j in range(4):
                    xt = stage.tile([P, D], FP32, name=f"x_{grp}_{j}", tag="x_sb",
                                    bufs=4)
                    nc.sync.dma_start(xt, x_d[(grp * 4 + j) * P:(grp * 4 + j + 1) * P, :])
                    xs.append(xt)
                for m in range(MC):
                    ps = psA.tile([P, 512], FP32, name=f"trp_{grp}_{m}", tag="ps")
                    for j in range(4):
                        nc.tensor.transpose(ps[:, j * P:(j + 1) * P],
                                            xs[j][:, m * P:(m + 1) * P], ident)
                    nc.scalar.activation(xT[m][:, grp * 512:(grp + 1) * 512], ps,
                                         AF.Copy)

            # weights -> bf16
            for w_d, w_bf, nm in ((wq_d, wq_bf, "q"), (wk_d, wk_bf, "k"),
                                  (wv_d, wv_bf, "v")):
                for m in range(MC):
                    wf = stage.tile([P, NHD], FP32, name=f"wf{nm}{m}", tag="wf32",
                                    bufs=4)
                    nc.sync.dma_start(wf, w_d[m * P:(m + 1) * P, :])
                    nc.vector.tensor_copy(w_bf[m], wf)
            for m in range(4):
                wf = stage.tile([P, D], FP32, name=f"wfo{m}", tag="wof32", bufs=2)
                nc.sync.dma_start(wf, wo_d[m * P:(m + 1) * P, :])
                nc.vector.tensor_copy(wo_bf[m], wf)

            # QT/KT: [512(d), T] over 4 d-chunks; scale 1/8 and bias folded in
            for w_bf, out_t, bias_t, scl in ((wq_bf, QT, biasq, 0.125),
                                             (wk_bf, KT, biask, 1.0)):
                for dc in range(4):
                    for qq in range(4):
                        ps = psA.tile([P, 512], FP32, name=f"pj{scl}_{dc}_{qq}",
                                      tag="ps")
                        for m in range(MC):
                            nc.tensor.matmul(ps,
                                             lhsT=w_bf[m][:, dc * P:(dc + 1) * P],
                                             rhs=xT[m][:, qq * 512:(qq + 1) * 512],
                                             start=(m == 0), stop=(m == MC - 1))
                        nc.scalar.activation(out_t[dc][:, qq * 512:(qq + 1) * 512],
                                             ps, AF.Identity,
                                             bias=bias_t[:, dc:dc + 1], scale=scl)
            # V: [T, 512] over 16 t-chunks
            for ti in range(TQ):
                ps = psA.tile([P, NHD], FP32, name=f"pv{ti}", tag="ps")
                for m in range(MC):
                    nc.tensor.matmul(ps, lhsT=xT[m][:, ti * P:(ti + 1) * P],
                                     rhs=wv_bf[m], start=(m == 0), stop=(m == MC - 1))
                nc.vector.tensor_copy(Vt[ti], ps)

        # ---------------- pass 1: scores, exp, Z, spill E, column sums c --
        with ExitStack() as p1:
            Epool = p1.enter_context(tc.tile_pool(name="Epool", bufs=20))
            psS = p1.enter_context(tc.tile_pool(name="psS", bufs=2, space="PSUM"))
            psC = p1.enter_context(tc.tile_pool(name="psC", bufs=1, space="PSUM"))
            c_ps = psC.tile([1, T], FP32, name="c_ps", tag="c_ps")

            for h in range(NH):
                dc, ro = h // 2, (h % 2) * HD
                Etiles = []
                for qc in range(TQ):
                    E_sb = Epool.tile([P, T], BF16, name=f"E_{h}_{qc}", tag="E")
                    for sh in range(2):
                        ps = psS.tile([P, 1024], FP32, name=f"S_{h}_{qc}_{sh}",
                                      tag="S")
                        for kq in range(2):
                            nc.tensor.matmul(
                                ps[:, kq * 512:(kq + 1) * 512],
                                lhsT=QT[dc][ro:ro + HD, qc * P:(qc + 1) * P],
                                rhs=KT[dc][ro:ro + HD,
                                           sh * 1024 + kq * 512:
                                           sh * 1024 + (kq + 1) * 512],
                                start=True, stop=True)
                        nc.scalar.activation(
                            E_sb[:, sh * 1024:(sh + 1) * 1024], ps, AF.Exp,
                            accum_out=Zh[h][:, qc * 2 + sh:qc * 2 + sh + 1])
                    nc.sync.dma_start(
                        E_spill[h * T + qc * P:h * T + (qc + 1) * P, :], E_sb)
                    Etiles.append(E_sb)
                # r = 1 / rowsum(E)
                nc.vector.tensor_add(rV[h], Zh[h][:, 0:2 * TQ:2],
                                     Zh[h][:, 1:2 * TQ:2])
                nc.vector.reciprocal(rV[h], rV[h])
                nc.vector.tensor_copy(rB[h], rV[h])
                # c += E^T r  (per q-chunk vector matmuls, accumulate in PSUM)
                for qc in range(TQ):
                    for kq in range(4):
                        nc.tensor.matmul(c_ps[0:1, kq * 512:(kq + 1) * 512],
                                         lhsT=rB[h][:, qc:qc + 1],
                                         rhs=Etiles[qc][:, kq * 512:(kq + 1) * 512],
                                         start=(h == 0 and qc == 0),
                                         stop=(h == NH - 1 and qc == TQ - 1))

            nc.scalar.activation(c_sb, c_ps, AF.Copy, scale=1.0 / (H * T))
            nc.sync.dma_start(c_bounce, c_sb)
            nc.gpsimd.collective_compute(
                "AllReduce", mybir.AluOpType.add,
                replica_groups=[[0, 1], [2, 3], [4, 5], [6, 7]],
                ins=[c_bounce[:].opt()], outs=[w_bounce[:].opt()])
            nc.sync.dma_start(
                w_col, w_bounce[:].rearrange("a (c p) -> (a p) c", p=P))

        # ---------------- pass 2: u = E^T (w*r), pooled = u^T V, @ Wo -----
        with ExitStack() as p2:
            E2pool = p2.enter_context(tc.tile_pool(name="E2pool", bufs=8))
            small = p2.enter_context(tc.tile_pool(name="small", bufs=2))
            psU = p2.enter_context(tc.tile_pool(name="psU", bufs=1, space="PSUM"))
            psP = p2.enter_context(tc.tile_pool(name="psP", bufs=1, space="PSUM"))
            pooled_ps = psP.tile([1, NHD], FP32, name="pooled_ps", tag="pooled_ps")

            for h in range(NH):
                gf = small.tile([P, TQ], FP32, name=f"gf{h}", tag="gf")
                nc.vector.tensor_mul(gf, w_col, rV[h])
                nc.vector.tensor_copy(gB[h], gf)
                u_ps = psU.tile([1, T], FP32, name=f"u_ps{h}", tag="u_ps")
                for qc in range(TQ):
                    E2 = E2pool.tile([P, T], BF16, name=f"E2_{h}_{qc}", tag="E2")
                    nc.sync.dma_start(
                        E2, E_spill[h * T + qc * P:h * T + (qc + 1) * P, :])
                    for kq in range(4):
                        nc.tensor.matmul(u_ps[0:1, kq * 512:(kq + 1) * 512],
                                         lhsT=gB[h][:, qc:qc + 1],
                                         rhs=E2[:, kq * 512:(kq + 1) * 512],
                                         start=(qc == 0), stop=(qc == TQ - 1))
                u_sb = small.tile([1, T], BF16, name=f"u_sb{h}", tag="u_sb")
                nc.scalar.activation(u_sb, u_ps, AF.Copy)
                nc.sync.dma_start(u_dram[h:h + 1, :], u_sb)
                u_col = small.tile([P, TQ], BF16, name=f"u_col{h}", tag="u_col")
                nc.sync.dma_start(
                    u_col,
                    u_dram[h:h + 1, :].rearrange("a (c p) -> (a p) c", p=P))
                for kc in range(TQ):
                    nc.tensor.matmul(pooled_ps[0:1, h * HD:(h + 1) * HD],
                                     lhsT=u_col[:, kc:kc + 1],
                                     rhs=Vt[kc][:, h * HD:(h + 1) * HD],
                                     start=(kc == 0), stop=(kc == TQ - 1))

            pooled_sb = small.tile([1, NHD], BF16, name="pooled_sb", tag="pooled_sb")
            nc.scalar.activation(pooled_sb, pooled_ps, AF.Copy)
            nc.sync.dma_start(pooled_dram, pooled_sb)
            pooled_col = small.tile([P, 4], BF16, name="pooled_col", tag="pooled_col")
            nc.sync.dma_start(
                pooled_col, pooled_dram[:].rearrange("a (c p) -> (a p) c", p=P))

            part_ps = psU.tile([1, D], FP32, name="part_ps", tag="part_ps")
            for mc in range(4):
                for hf in range(2):
                    nc.tensor.matmul(part_ps[0:1, hf * 512:(hf + 1) * 512],
                                     lhsT=pooled_col[:, mc:mc + 1],
                                     rhs=wo_bf[mc][:, hf * 512:(hf + 1) * 512],
                                     start=(mc == 0), stop=(mc == 3))
            out_sb = small.tile([1, D], FP32, name="out_sb", tag="out_sb")
            nc.scalar.activation(out_sb, part_ps, AF.Copy)
            nc.sync.dma_start(out_d[:], out_sb)


_NC_CACHE = None


def build_nc():
    global _NC_CACHE
    if _NC_CACHE is not None:
        return _NC_CACHE
    nc = bacc.Bacc("TRN2", target_bir_lowering=False, debug=False,
                   enable_asserts=False, num_devices=8)
    x_d = nc.dram_tensor("x", [T, D], FP32, kind="ExternalInput")
    wq_d = nc.dram_tensor("wq", [D, NHD], FP32, kind="ExternalInput")
    wk_d = nc.dram_tensor("wk", [D, NHD], FP32, kind="ExternalInput")
    wv_d = nc.dram_tensor("wv", [D, NHD], FP32, kind="ExternalInput")
    wo_d = nc.dram_tensor("wo", [NHD, D], FP32, kind="ExternalInput")
    bqs_d = nc.dram_tensor("bqs", [NHD], FP32, kind="ExternalInput")
    bkc_d = nc.dram_tensor("bkc", [NHD], FP32, kind="ExternalInput")
    out_d = nc.dram_tensor("out", [1, D], FP32, kind="ExternalOutput")
    with tile.TileContext(nc) as tc:
        _body(tc, x_d.ap(), wq_d.ap(), wk_d.ap(), wv_d.ap(), wo_d.ap(),
              bqs_d.ap(), bkc_d.ap(), out_d.ap())
    nc.compile()
    _NC_CACHE = nc
    return nc


def make_in_maps(x, Wq, bq, Wk, bk, Wv, bv, Wo, bo):
    in_maps = []
    for core in range(8):
        b, g = core // 2, core % 2
        cs = slice(g * NHD, (g + 1) * NHD)
        in_maps.append({
            "x": np.ascontiguousarray(x[b]),
            "wq": np.ascontiguousarray(Wq[:, cs]),
            "wk": np.ascontiguousarray(Wk[:, cs]),
            "wv": np.ascontiguousarray(Wv[:, cs]),
            "wo": np.ascontiguousarray(Wo[cs, :]),
            "bqs": np.ascontiguousarray(bq[cs]) * np.float32(0.125),
            "bkc": np.ascontiguousarray(bk[cs]),
        })
    return in_maps


def kernel(x, Wq, bq, Wk, bk, Wv, bv, Wo, bo, _results_hook=None):
    x, Wq, bq, Wk, bk, Wv, bv, Wo, bo = (
        np.asarray(a, dtype=np.float32)
        for a in (x, Wq, bq, Wk, bk, Wv, bv, Wo, bo))
    nc = build_nc()
    in_maps = make_in_maps(x, Wq, bq, Wk, bk, Wv, bv, Wo, bo)
    res = run_bass_kernel_spmd(nc, in_maps, core_ids=list(range(8)))
    if _results_hook is not None:
        _results_hook(res)
    parts = [res.results[c]["out"][0] for c in range(8)]
    correction = bv.astype(np.float32) @ Wo.astype(np.float32) + bo
    out = np.stack([parts[2 * b] + parts[2 * b + 1] for b in range(B)])
    return (out + correction[None, :]).astype(np.float32)
